# revision 1
# baseline (speedup 1.0000x reference)
"""H2GCN forward on 8 Trainium2 NeuronCores.

out = concat([h0, A1@h0, A2@h0], 1) @ W_out + b_out,  h0 = x @ W1

Data-parallel over destination nodes (1250 rows/core, padded to 1280).
Per core: h0 shard GEMM (fp32r matmuls fed from one blob DMA so each PE
instruction carries at most one sync wait), AllGather of h0 in bf16,
SpMM as dma_gather of source rows + 128-edge selection-matrix bf16
matmuls accumulated in PSUM per dest tile, PE transposes to
feature-major, final fp32r GEMM with the bias applied as a K=1 matmul.
"""
import sys
import types

for _p in ("/opt/trn_rl_repo", "/root/.axon_site", "/root/.axon_site/_ro/trn_rl_repo",
           "/root/.axon_site/_ro/pypackages"):
    if _p not in sys.path:
        sys.path.append(_p)

import numpy as np
import ml_dtypes
import concourse.bass as bass
import concourse.bacc as bacc
import concourse.mybir as mybir
import concourse.tile as tile
from concourse import bass_utils

N, IN_C, HID, OUT_C = 10000, 2048, 256, 256
NCORES = 8
ROWS = N // NCORES          # 1250
PROWS = 1280                # padded (10 x 128)
NT = PROWS // 128
KT = IN_C // 128
ST = NCORES * (PROWS // 128)   # 80 source tiles in the padded AllGather space

f32 = mybir.dt.float32
f32r = mybir.dt.float32r
bf16 = mybir.dt.bfloat16
i16 = mybir.dt.int16
bfnp = ml_dtypes.bfloat16

# blob_a: xT k-tiles then W1 k-tiles (f32 elements per partition row)
OXA, OW1 = 0, KT * PROWS
BLOBA = KT * PROWS + KT * HID
# blob_b: Wout k-tiles, bias (row 0), ones (row 0), identity
OWO, OB, OO, OI = 0, 6 * OUT_C, 6 * OUT_C + OUT_C, 6 * OUT_C + OUT_C + 128
BLOBB = OI + 128

LAST_EXEC_NS = None
LAST_RESULTS = None


def _install_trace_shim():
    try:
        import antenv.axon_hooks  # noqa: F401
        return
    except ImportError:
        pass
    try:
        import antenv
        from trn_agent_boot.trn_boot import _ntff_profile_via_ctypes
        hook = _ntff_profile_via_ctypes("/opt/axon/libaxon_pjrt.so")
        mod = types.ModuleType("antenv.axon_hooks")
        mod.get_axon_ntff_profile_hook = lambda: hook
        mod.set_axon_ntff_profile_hook = lambda h: None
        sys.modules["antenv.axon_hooks"] = mod
        antenv.axon_hooks = mod
    except Exception:
        pass


def _dense_adj(rows, cols, vals, core):
    """Dense padded A^T for this core's dest shard, tiled [128, NT*ST*128]
    bf16 with block (dt, st) at columns (dt*ST+st)*128."""
    lo, hi = core * ROWS, (core + 1) * ROWS
    m = (rows >= lo) & (rows < hi)
    r, c, v = rows[m] - lo, cols[m], vals[m]
    A = np.zeros((NCORES * PROWS, PROWS), np.float32)
    src = (c // ROWS) * PROWS + (c % ROWS)
    np.add.at(A, (src, r), v)
    return np.ascontiguousarray(
        A.reshape(ST, 128, NT, 128).transpose(1, 2, 0, 3)
        .reshape(128, NT * ST * 128)).astype(bfnp)


def _build():
    nc = bacc.Bacc("TRN2", target_bir_lowering=False, debug=False,
                   num_devices=8)
    blob_a = nc.dram_tensor("blob_a", [128, BLOBA], f32r, kind="ExternalInput")
    blob_b = nc.dram_tensor("blob_b", [128, BLOBB], f32r, kind="ExternalInput")
    A1 = nc.dram_tensor("A1", [128, NT * ST * 128], bf16, kind="ExternalInput")
    A2 = nc.dram_tensor("A2", [128, NT * ST * 128], bf16, kind="ExternalInput")
    out = nc.dram_tensor("out", [ROWS, OUT_C], f32, kind="ExternalOutput")

    with tile.TileContext(nc) as tc:
        with tc.tile_pool(name="keep", bufs=1) as keep, \
             tc.tile_pool(name="dram", bufs=1, space="DRAM") as dram, \
             tc.tile_pool(name="pmm", bufs=2, space="PSUM") as pmm, \
             tc.tile_pool(name="psm", bufs=2, space="PSUM") as psm, \
             tc.tile_pool(name="ptr", bufs=2, space="PSUM") as ptr:

            h_sb = keep.tile([128, 3, NT, HID], f32)
            hT = keep.tile([128, 6, PROWS], f32r)
            blob_b_t = keep.tile([128, BLOBB], f32r)
            ident_v = keep.tile([128, 128], f32)
            ag_sb = keep.tile([128, NT, HID], bf16)
            nc.sync.dma_start(blob_b_t[:], blob_b[:])
            # identity produced on DVE so transposes need only one DVE wait
            nc.vector.tensor_copy(ident_v[:], blob_b_t[:, OI:OI + 128].bitcast(f32))

            HT = NT // 2
            ag_in0 = dram.tile([HT * 128, HID], bf16)
            ag_in1 = dram.tile([HT * 128, HID], bf16)
            ag_out0 = dram.tile([NCORES * HT * 128, HID], bf16,
                                addr_space="Shared")
            ag_out1 = dram.tile([NCORES * HT * 128, HID], bf16,
                                addr_space="Shared")

            # ---- phase A: h0 = x @ W1 (local shard), fp32r ----
            with nc.named_scope("h0_gemm"):
                with tc.tile_pool(name="pa", bufs=1) as pa:
                    blob_a_t = pa.tile([128, BLOBA], f32r)
                    qa = BLOBA // 4
                    for q in range(4):
                        nc.sync.dma_start(blob_a_t[:, q * qa:(q + 1) * qa],
                                          blob_a[:, q * qa:(q + 1) * qa])
                    for t in range(NT):
                        ps = pmm.tile([128, HID], f32, tag="mm")
                        for k in range(KT):
                            nc.tensor.matmul(
                                ps[:],
                                blob_a_t[:, OXA + k * PROWS + 128 * t:
                                         OXA + k * PROWS + 128 * (t + 1)],
                                blob_a_t[:, OW1 + k * HID:OW1 + (k + 1) * HID],
                                start=(k == 0), stop=(k == KT - 1),
                            )
                        nc.vector.tensor_copy(h_sb[:, 0, t, :], ps[:])
                        nc.vector.tensor_copy(ag_sb[:, t, :], ps[:])
                        if t == HT - 1:
                            nc.sync.dma_start(
                                ag_in0[:].rearrange("(a p) m -> p a m", p=128),
                                ag_sb[:, 0:HT, :])
                        if t == NT - 1:
                            nc.sync.dma_start(
                                ag_in1[:].rearrange("(a p) m -> p a m", p=128),
                                ag_sb[:, HT:NT, :])

            # ---- phase B: AllGather h0 (bf16), two halves ----
            with nc.named_scope("allgather"):
                nc.gpsimd.collective_compute(
                    "AllGather", mybir.AluOpType.bypass,
                    replica_groups=[list(range(NCORES))],
                    ins=[ag_in0.opt()], outs=[ag_out0.opt()],
                )
                nc.gpsimd.collective_compute(
                    "AllGather", mybir.AluOpType.bypass,
                    replica_groups=[list(range(NCORES))],
                    ins=[ag_in1.opt()], outs=[ag_out1.opt()],
                )

            # ---- phase C: SpMM as dense-block matmuls vs resident h0 ----
            with nc.named_scope("spmm"):
                with tc.tile_pool(name="pc", bufs=1) as pc:
                    h0a = pc.tile([128, ST, HID], bf16)
                    for r in range(NCORES):
                        nc.sync.dma_start(
                            h0a[:, r * NT:r * NT + HT, :],
                            ag_out0[r * HT * 128:(r + 1) * HT * 128, :]
                            .rearrange("(t p) m -> p t m", p=128))
                        nc.sync.dma_start(
                            h0a[:, r * NT + HT:(r + 1) * NT, :],
                            ag_out1[r * HT * 128:(r + 1) * HT * 128, :]
                            .rearrange("(t p) m -> p t m", p=128))
                    st_order = [s for s in range(ST) if s % NT < HT] + \
                               [s for s in range(ST) if s % NT >= HT]
                    for a, A_d in enumerate([A1, A2]):
                        for t in range(NT):
                            a_t = pc.tile([128, ST * 128], bf16, tag="a",
                                          bufs=3)
                            nc.sync.dma_start(
                                a_t[:],
                                A_d[:, t * ST * 128:(t + 1) * ST * 128])
                            ps = psm.tile([128, HID], f32, tag="smm")
                            for i, st in enumerate(st_order):
                                nc.tensor.matmul(
                                    ps[:], a_t[:, 128 * st:128 * (st + 1)],
                                    h0a[:, st, :],
                                    start=(i == 0), stop=(i == ST - 1),
                                )
                            nc.vector.tensor_copy(h_sb[:, 1 + a, t, :], ps[:])

            # ---- phase D: transpose h -> feature-major ----
            with nc.named_scope("transpose"):
                for part in range(3):
                    for t in range(NT):
                        for half in range(2):
                            pst = ptr.tile([128, 128], f32, tag="tr")
                            nc.tensor.transpose(
                                pst[:],
                                h_sb[:, part, t, 128 * half:128 * (half + 1)],
                                ident_v[:],
                            )
                            nc.vector.tensor_copy(
                                hT[:, 2 * part + half, 128 * t:128 * (t + 1)],
                                pst[:])

            # ---- phase E: out = h @ Wout + b (fp32r) ----
            with nc.named_scope("out_gemm"):
                for t in range(NT):
                    ps = pmm.tile([128, OUT_C], f32, tag="mm")
                    nc.tensor.matmul(ps[:], blob_b_t[0:1, OO:OO + 128],
                                     blob_b_t[0:1, OB:OB + OUT_C],
                                     start=True, stop=False)
                    for k in range(6):
                        nc.tensor.matmul(
                            ps[:],
                            hT[:, k, 128 * t:128 * (t + 1)],
                            blob_b_t[:, OWO + k * OUT_C:OWO + (k + 1) * OUT_C],
                            start=False, stop=(k == 5),
                        )
                    o_sb = keep.tile([128, OUT_C], f32, tag="osb", bufs=2)
                    nc.vector.tensor_copy(o_sb[:], ps[:])
                    rows = min(128, ROWS - 128 * t)
                    nc.sync.dma_start(out[128 * t:128 * t + rows, :],
                                      o_sb[:rows, :])
    nc.compile()
    return nc


def kernel(x, adj1_rows, adj1_cols, adj1_vals, adj2_rows, adj2_cols, adj2_vals,
           W1, W_out, b_out):
    global LAST_EXEC_NS, LAST_RESULTS
    _install_trace_shim()
    x = np.asarray(x, np.float32)
    W1 = np.ascontiguousarray(np.asarray(W1, np.float32))
    W_out = np.ascontiguousarray(np.asarray(W_out, np.float32))
    b_out = np.asarray(b_out, np.float32).ravel()

    w1_cols = W1.reshape(KT, 128, HID).transpose(1, 0, 2).reshape(128, KT * HID)
    blob_b = np.zeros((128, BLOBB), np.float32)
    blob_b[:, OWO:OWO + 6 * OUT_C] = \
        W_out.reshape(6, 128, OUT_C).transpose(1, 0, 2).reshape(128, 6 * OUT_C)
    blob_b[0, OB:OB + OUT_C] = b_out
    blob_b[0, OO:OO + 128] = 1.0
    blob_b[:, OI:OI + 128] = np.eye(128, dtype=np.float32)

    in_maps = []
    for c in range(NCORES):
        xtp = np.zeros((IN_C, PROWS), np.float32)
        xtp[:, :ROWS] = x[c * ROWS:(c + 1) * ROWS].T
        blob_a = np.concatenate([
            xtp.reshape(KT, 128, PROWS).transpose(1, 0, 2).reshape(128, KT * PROWS),
            w1_cols,
        ], axis=1)
        in_maps.append({
            "blob_a": blob_a, "blob_b": blob_b,
            "A1": _dense_adj(np.asarray(adj1_rows, np.int64),
                             np.asarray(adj1_cols, np.int64),
                             np.asarray(adj1_vals, np.float32), c),
            "A2": _dense_adj(np.asarray(adj2_rows, np.int64),
                             np.asarray(adj2_cols, np.int64),
                             np.asarray(adj2_vals, np.float32), c),
        })

    nc = _build()
    try:
        res = bass_utils.run_bass_kernel_spmd(
            nc, in_maps, core_ids=list(range(NCORES)), trace=True,
            trace_cores=[0])
    except Exception:
        res = bass_utils.run_bass_kernel_spmd(
            nc, in_maps, core_ids=list(range(NCORES)), trace=False)
    LAST_EXEC_NS = res.exec_time_ns
    LAST_RESULTS = res
    return np.concatenate([res.results[c]["out"] for c in range(NCORES)], axis=0)



# revision 11
# speedup vs baseline: 1.5733x; 1.5733x over previous
"""H2GCN forward on 8 Trainium2 NeuronCores — dense fp8 DoubleRow SpMM.

out = concat([h0, A1@h0, A2@h0], 1) @ W_out + b_out,  h0 = x @ W1

Data-parallel over destination nodes (1250 rows/core). Per core:
  h0 = x_shard @ W1 in bf16 (t-major, xT k-tiles streamed from DRAM),
  AllGather h0 in fp8e4 (two halves: rows 0-511 / 512-1249),
  SpMM as dense-block matmuls in fp8 DoubleRow perf mode: each instruction
  contracts a PAIR of 128-row source tiles (256 rows) against the dest-tile
  selection block at 0.5 cycles/row — 4x the bf16 dense rate, and the A
  matrices are 26 MB instead of 52 MB.  Half-0/half-1 source partials are
  kept separate (hT k-slices 2-5 / 6-9) so half-0 matmuls run while the
  second AllGather is still in flight; the final out GEMM sums both.
"""
import sys
import types

for _p in ("/opt/trn_rl_repo", "/root/.axon_site", "/root/.axon_site/_ro/trn_rl_repo",
           "/root/.axon_site/_ro/pypackages"):
    if _p not in sys.path:
        sys.path.append(_p)

import numpy as np
import ml_dtypes
import concourse.bass as bass
import concourse.bacc as bacc
import concourse.mybir as mybir
import concourse.tile as tile
from concourse import bass_utils

N, IN_C, HID, OUT_C = 10000, 2048, 256, 256
NCORES = 8
ROWS = N // NCORES          # 1250
NT = 10                     # dest tiles of 128 (last has 98 valid rows)
KT = IN_C // 128            # 16
H0ROWS = 512                # AG half 0: source tiles 0-3 of each shard
H1ROWS = 768                # half 1: tiles 4-9 incl. zero pad rows 1250-1279
NPAIR = 5                   # source-tile pairs per shard (DoubleRow)
WV = 32                     # chunks per A-stream wave

f32 = mybir.dt.float32
bf16 = mybir.dt.bfloat16
f8 = mybir.dt.float8e4
bfnp = ml_dtypes.bfloat16
f8np = ml_dtypes.float8_e4m3

# small weights blob (bf16): Wout k-tiles, bias, ones, identity
OWO, OB, OO, OI = 0, 6 * OUT_C, 6 * OUT_C + OUT_C, 6 * OUT_C + OUT_C + 128
BLOBW = OI + 128

LAST_EXEC_NS = None
LAST_RESULTS = None


def _install_trace_shim():
    try:
        import antenv.axon_hooks  # noqa: F401
        return
    except ImportError:
        pass
    try:
        import antenv
        from trn_agent_boot.trn_boot import _ntff_profile_via_ctypes
        hook = _ntff_profile_via_ctypes("/opt/axon/libaxon_pjrt.so")
        mod = types.ModuleType("antenv.axon_hooks")
        mod.get_axon_ntff_profile_hook = lambda: hook
        mod.set_axon_ntff_profile_hook = lambda h: None
        sys.modules["antenv.axon_hooks"] = mod
        antenv.axon_hooks = mod
    except Exception:
        pass


def _chunk_meta():
    """Global chunk order: half-major, dest-tile, adj, then (core, pair).
    half 0 = pairs 0-1 (src rows 0-511), half 1 = pairs 2-4 (rows 512-1279).
    Returns (meta, tile_last): meta[i] = (a, t, h, first, last)."""
    meta = []
    tile_last = {}
    for h in (0, 1):
        pairs = [0, 1] if h == 0 else [2, 3, 4]
        for t in range(NT):
            for a in (0, 1):
                n = NCORES * len(pairs)
                for i in range(n):
                    meta.append((a, t, h, i == 0, i == n - 1))
                    tile_last[t] = len(meta) - 1
    return meta, tile_last


def _prep_xt(x_bf, core):
    """xT blob: block (t, k) at cols (t*KT + k)*128, [128 feat, 128 rows]."""
    xt = np.zeros((128, NT * KT * 128), bfnp)
    xsh = x_bf[core * ROWS:(core + 1) * ROWS]
    for t in range(NT):
        rows = min(128, ROWS - t * 128)
        b = xsh[t * 128:t * 128 + rows].T.reshape(KT, 128, rows)
        for k in range(KT):
            xt[:, (t * KT + k) * 128:(t * KT + k) * 128 + rows] = b[k]
    return xt


def _prep_adj(adj, core):
    """Dense A^T per adjacency for this core's dest shard in fp8, laid out in
    global chunk order: chunk = (a,t,h,core r,pair j2) -> [128, 2, 128]."""
    dense = []
    for (rows, cols, vals) in adj:
        lo = core * ROWS
        m = (rows >= lo) & (rows < lo + ROWS)
        r, c, v = rows[m] - lo, cols[m], vals[m]
        # src index in padded tile space: core*1280 + (col % 1250)
        src = (c // ROWS) * (NT * 128) + (c % ROWS)
        A = np.zeros((NCORES * NT * 128, ROWS), np.float32)
        np.add.at(A, (src, r), v)
        dense.append(A.astype(f8np))
    meta, _ = _chunk_meta()
    blob = np.zeros((128, len(meta) * 256), f8np)
    pos = {}
    cnt = {}
    for i, (a, t, h, _, _) in enumerate(meta):
        k = (a, t, h)
        j = cnt.get(k, 0)
        cnt[k] = j + 1
        pos[(a, t, h, j)] = i
    for a in (0, 1):
        A = dense[a]
        for h in (0, 1):
            pairs = [0, 1] if h == 0 else [2, 3, 4]
            for t in range(NT):
                for rj, (rr, j2) in enumerate(
                        (rr, j2) for rr in range(NCORES) for j2 in pairs):
                    i = pos[(a, t, h, rj)]
                    for half_pair in range(2):
                        s = rr * NT + 2 * j2 + half_pair
                        blk = A[s * 128:(s + 1) * 128,
                                t * 128:min((t + 1) * 128, ROWS)]
                        blob[:, i * 256 + half_pair * 128:
                             i * 256 + half_pair * 128 + blk.shape[1]] = blk
    return blob


def _build():
    meta, tile_last = _chunk_meta()
    nchunks = len(meta)
    nc = bacc.Bacc("TRN2", target_bir_lowering=False, debug=False,
                   num_devices=NCORES)
    xt_d = nc.dram_tensor("xt", [128, NT * KT * 128], bf16, kind="ExternalInput")
    w1_d = nc.dram_tensor("w1", [128, KT * HID], bf16, kind="ExternalInput")
    ws_d = nc.dram_tensor("ws", [128, BLOBW], bf16, kind="ExternalInput")
    a_d = nc.dram_tensor("ablob", [128, nchunks * 256], f8, kind="ExternalInput")
    out = nc.dram_tensor("out", [ROWS, OUT_C], f32, kind="ExternalOutput")

    with tile.TileContext(nc) as tc:
        with tc.tile_pool(name="keep", bufs=1) as keep, \
             tc.tile_pool(name="dram", bufs=1, space="DRAM") as dram, \
             tc.tile_pool(name="pmm", bufs=2, space="PSUM") as pmm, \
             tc.tile_pool(name="psm", bufs=3, space="PSUM") as psm, \
             tc.tile_pool(name="ptr", bufs=2, space="PSUM") as ptr:

            w1_sb = keep.tile([128, KT * HID], bf16)
            ws_sb = keep.tile([128, BLOBW], bf16)
            ag_sb = keep.tile([128, NT, HID], f8)
            h0a = keep.tile([128, NCORES * NT, HID], f8)
            # hT k-slices: 0,1 = h0; 2..5 = h1,h2 half-0 partials;
            # 6..9 = h1,h2 half-1 partials (out GEMM sums both)
            hT = keep.tile([128, 10, NT * 128], bf16)
            h12 = keep.tile([128, 2, NT, HID], bf16)
            nc.sync.dma_start(w1_sb[:], w1_d[:])
            nc.sync.dma_start(ws_sb[:], ws_d[:])

            ag_in0 = dram.tile([H0ROWS, HID], f8)
            ag_in1 = dram.tile([H1ROWS, HID], f8)
            ag_out0 = dram.tile([NCORES * H0ROWS, HID], f8, addr_space="Shared")
            ag_out1 = dram.tile([NCORES * H1ROWS, HID], f8, addr_space="Shared")

            # ---- phase A: h0 = x @ W1 (bf16), t-major with streamed xT ----
            with nc.named_scope("h0_gemm"):
                for t in range(NT):
                    xtile = keep.tile([128, KT * 128], bf16, tag="xt", bufs=2)
                    nc.sync.dma_start(
                        xtile[:], xt_d[:, t * KT * 128:(t + 1) * KT * 128])
                    ps = pmm.tile([128, HID], f32, tag="mm")
                    for k in range(KT):
                        nc.tensor.matmul(
                            ps[:], xtile[:, k * 128:(k + 1) * 128],
                            w1_sb[:, k * HID:(k + 1) * HID],
                            start=(k == 0), stop=(k == KT - 1))
                    h0bf = keep.tile([128, HID], bf16, tag="h0bf", bufs=2)
                    nc.vector.tensor_copy(h0bf[:], ps[:])
                    nc.vector.tensor_copy(ag_sb[:, t, :], ps[:])
                    for hf in range(2):
                        pst = ptr.tile([128, 128], bf16, tag="tr")
                        nc.tensor.transpose(
                            pst[:], h0bf[:, 128 * hf:128 * (hf + 1)],
                            ws_sb[:, OI:OI + 128])
                        nc.vector.tensor_copy(
                            hT[:, hf, 128 * t:128 * (t + 1)], pst[:])
                    if t == 3:
                        nc.sync.dma_start(
                            ag_in0[:].rearrange("(a p) m -> p a m", p=128),
                            ag_sb[:, 0:4, :])
                    if t == NT - 1:
                        nc.sync.dma_start(
                            ag_in1[:].rearrange("(a p) m -> p a m", p=128),
                            ag_sb[:, 4:10, :])

            # ---- phase B: AllGather h0 (fp8), two halves ----
            with nc.named_scope("allgather"):
                nc.gpsimd.collective_compute(
                    "AllGather", mybir.AluOpType.bypass,
                    replica_groups=[list(range(NCORES))],
                    ins=[ag_in0.opt()], outs=[ag_out0.opt()])
                nc.gpsimd.collective_compute(
                    "AllGather", mybir.AluOpType.bypass,
                    replica_groups=[list(range(NCORES))],
                    ins=[ag_in1.opt()], outs=[ag_out1.opt()])

            # ---- phase C: dense SpMM, fp8 DoubleRow, streamed A ----
            with nc.named_scope("spmm"):
                for r in range(NCORES):
                    nc.sync.dma_start(
                        h0a[:, r * NT:r * NT + 4, :],
                        ag_out0[r * H0ROWS:(r + 1) * H0ROWS, :]
                        .rearrange("(t p) m -> p t m", p=128))
                    nc.sync.dma_start(
                        h0a[:, r * NT + 4:(r + 1) * NT, :],
                        ag_out1[r * H1ROWS:(r + 1) * H1ROWS, :]
                        .rearrange("(t p) m -> p t m", p=128))

                cur_ps = {}
                srcpair = {}
                cnt = {}
                for i, (a, t, h, _, _) in enumerate(meta):
                    pairs = [0, 1] if h == 0 else [2, 3, 4]
                    j = cnt.get((a, t, h), 0)
                    cnt[(a, t, h)] = j + 1
                    rr, j2 = j // len(pairs), pairs[j % len(pairs)]
                    srcpair[i] = rr * NPAIR + j2
                ci = 0
                while ci < len(meta):
                    wn = min(WV, len(meta) - ci)
                    # don't let a wave cross the half boundary
                    hcur = meta[ci][2]
                    while meta[ci + wn - 1][2] != hcur:
                        wn -= 1
                    at = keep.tile([128, WV, 2, 128], f8, tag="a", bufs=3)
                    nc.sync.dma_start(
                        at[:, 0:wn, :, :].rearrange("p w i d -> p (w i d)"),
                        a_d[:, ci * 256:(ci + wn) * 256])
                    for j in range(wn):
                        a, t, h, first, last = meta[ci + j]
                        if first:
                            cur_ps[(a, t)] = psm.tile(
                                [128, HID], f32, tag="sc", name="scps")
                        sp = srcpair[ci + j]
                        nc.tensor.matmul(
                            cur_ps[(a, t)][:], at[:, j, :, :],
                            h0a[:, 2 * sp:2 * sp + 2, :],
                            perf_mode=mybir.MatmulPerfMode.DoubleRow,
                            start=first, stop=last)
                        if last:
                            nc.vector.tensor_copy(
                                h12[:, a, t, :], cur_ps[(a, t)][:])
                            for hf in range(2):
                                pst = ptr.tile([128, 128], bf16, tag="tr")
                                nc.tensor.transpose(
                                    pst[:],
                                    h12[:, a, t, 128 * hf:128 * (hf + 1)],
                                    ws_sb[:, OI:OI + 128])
                                nc.vector.tensor_copy(
                                    hT[:, 2 + 4 * h + 2 * a + hf,
                                       128 * t:128 * (t + 1)], pst[:])
                        if ci + j == tile_last[t]:
                            po = pmm.tile([128, OUT_C], f32, tag="mm")
                            nc.tensor.matmul(
                                po[:], ws_sb[0:1, OO:OO + 128],
                                ws_sb[0:1, OB:OB + OUT_C],
                                start=True, stop=False)
                            for i_k in range(10):
                                wk = i_k if i_k < 2 else 2 + (i_k - 2) % 4
                                nc.tensor.matmul(
                                    po[:], hT[:, i_k, 128 * t:128 * (t + 1)],
                                    ws_sb[:, OWO + wk * OUT_C:
                                          OWO + (wk + 1) * OUT_C],
                                    start=False, stop=(i_k == 9))
                            o_sb = keep.tile([128, OUT_C], f32, tag="osb", bufs=2)
                            nc.vector.tensor_copy(o_sb[:], po[:])
                            orows = min(128, ROWS - 128 * t)
                            nc.sync.dma_start(
                                out[128 * t:128 * t + orows, :], o_sb[:orows, :])
                    ci += wn
    nc.compile()
    return nc


def kernel(x, adj1_rows, adj1_cols, adj1_vals, adj2_rows, adj2_cols, adj2_vals,
           W1, W_out, b_out):
    global LAST_EXEC_NS, LAST_RESULTS
    _install_trace_shim()
    x_bf = np.ascontiguousarray(np.asarray(x, np.float32)).astype(bfnp)
    W1 = np.ascontiguousarray(np.asarray(W1, np.float32))
    W_out = np.ascontiguousarray(np.asarray(W_out, np.float32))
    b_out = np.asarray(b_out, np.float32).ravel()

    w1_blob = np.ascontiguousarray(
        W1.reshape(KT, 128, HID).transpose(1, 0, 2).reshape(128, KT * HID)
    ).astype(bfnp)
    ws = np.zeros((128, BLOBW), np.float32)
    ws[:, OWO:OWO + 6 * OUT_C] = \
        W_out.reshape(6, 128, OUT_C).transpose(1, 0, 2).reshape(128, 6 * OUT_C)
    ws[0, OB:OB + OUT_C] = b_out
    ws[0, OO:OO + 128] = 1.0
    ws[:, OI:OI + 128] = np.eye(128, dtype=np.float32)
    ws = ws.astype(bfnp)

    adj = [(np.asarray(adj1_rows, np.int64), np.asarray(adj1_cols, np.int64),
            np.asarray(adj1_vals, np.float32)),
           (np.asarray(adj2_rows, np.int64), np.asarray(adj2_cols, np.int64),
            np.asarray(adj2_vals, np.float32))]
    in_maps = []
    for c in range(NCORES):
        in_maps.append({
            "xt": _prep_xt(x_bf, c), "w1": w1_blob, "ws": ws,
            "ablob": _prep_adj(adj, c),
        })

    nc = _build()
    try:
        res = bass_utils.run_bass_kernel_spmd(
            nc, in_maps, core_ids=list(range(NCORES)), trace=True,
            trace_cores=[0])
    except Exception:
        res = bass_utils.run_bass_kernel_spmd(
            nc, in_maps, core_ids=list(range(NCORES)), trace=False)
    LAST_EXEC_NS = res.exec_time_ns
    LAST_RESULTS = res
    return np.concatenate([res.results[c]["out"] for c in range(NCORES)], axis=0)
